# revision 1
# baseline (speedup 1.0000x reference)
"""Trainium2 Bass kernel for a 2-layer GCN encoder with global mean pool.

Sharding: dst-partition of nodes across 8 NeuronCores (12500 nodes/core,
padded to 12544 slots = 98 blocks of 128). Both convs share ONE edge layout:
x is permuted host-side into the same table-row order that conv2's h1 table
uses (row = owner*12544 + block*128 + slot), so the chunk of an edge
(= src_core//2, int16 gather-index limit) and therefore the packed stream,
gather indices and one-hot metadata are identical for conv1 and conv2 and
are uploaded once. An LPT-greedy + swap-repair packing balances (block,
chunk) cells against a mixed 4/5-tile profile (b % 49 == 0 gets 5), leaving
<1% slot padding.

Each conv gathers bf16 source rows from a DRAM table via dma_gather (100
gathers per conv, one group prefetched ahead; the gather-index upload is
split per group and bulk constants are emitted mid-conv1 so the first
gather starts at ~7us), scatters each 128-edge tile into a [128,512] PSUM
super-block with TensorE matmuls against a bf16 one-hot ("valhot" =
(iota==dstslot) * rsqrt(deg_src)) built by one fused tensor_scalar (4x DVE
mode; GpSimd runs only gather preps). Self-loop messages are injected from
persistent SBUF copies of the local tables (pre-transposed x shard uploaded
once; conv1's ReLU output is written straight into an SBUF cache reused by
conv2) via a diag(dinv) matmul emitted after the edge matmuls. The
1/sqrt(deg_dst) scale, bias and ReLU are applied after a bf16 128x128 GEMM,
with ReLU + bf16 cast on the otherwise idle Scalar engine. h1 is
AllGather-ed in bf16 between the convs (half the f32 bytes); per-graph sums
ride a batch-id one-hot into PSUM and are combined with a small bf16
AllReduce before the two linear heads.

All floating-point math runs on device; the host only prepares integer
index/degree metadata, permuted/bf16-cast copies of inputs, and the packing.
"""
import sys

sys.path.insert(0, "/opt/trn_rl_repo")

import os as _osmod
import numpy as np
import ml_dtypes

KPOOLVH = int(_osmod.environ.get("KPOOLVH", "0"))     # 1/N of vh on gpsimd, 0=off
KPREFETCH = int(_osmod.environ.get("KPREFETCH", "1"))  # gather groups ahead
KMSGBUFS = int(_osmod.environ.get("KMSGBUFS", "10"))
KGG = int(_osmod.environ.get("KGG", "1"))              # super-blocks per gather group
KVHBUFS = int(_osmod.environ.get("KVHBUFS", "20"))
KWPBUFS = int(_osmod.environ.get("KWPBUFS", "6"))
KPHBUFS = int(_osmod.environ.get("KPHBUFS", "6"))
KXPBUFS = int(_osmod.environ.get("KXPBUFS", "4"))
KPREB = int(_osmod.environ.get("KPREB", "0"))   # conv2 groups with vh prebuilt
KAGGBUFS = int(_osmod.environ.get("KAGGBUFS", "3"))   # PSUM agg banks
KGEMBUFS = int(_osmod.environ.get("KGEMBUFS", "2"))   # PSUM gemm tiles

N = 100000
E = 1600000
G = 256
NCORES = 8
NSHARD = N // NCORES            # 12500 real nodes per core
NPAD = 12544                    # padded shard size (= 49*256 = 98*128)
BLK = int(_osmod.environ.get("KBLK", "128"))  # valhot/psum block width
NBLK = NPAD // BLK              # blocks per core
NSUB = NPAD // 128              # 98 GEMM sub-blocks per core
CH = 4                          # src chunks (int16 gather index limit)
W = 2 * NPAD                    # 25088 table rows per chunk window
# mixed per-block tile profile: every 3rd block gets one extra tile/cell
KT9 = int(_osmod.environ.get("KT9", "49"))  # b % KT9 == 0 -> big cell (0=all big)
_TBIG = (BLK * 9) // 256 + (1 if BLK < 256 else 0)   # 9 for 256, 5 for 128
TBLK = np.array([_TBIG if (KT9 == 0 or b % KT9 == 0) else _TBIG - 1
                 for b in range(NBLK)])    # tiles per (block, chunk) cell
CAP = TBLK * 128                # edge slots per cell
NTILES = int(TBLK.sum()) * CH   # tiles per conv per core
NSLOT = NTILES * 128            # edge slots per conv per core
PSB = 512 // BLK                # blocks per 512-wide psum super-block
SBS = [(s * PSB, PSB) for s in range(NBLK // PSB)]
if NBLK % PSB:
    SBS.append((NBLK - NBLK % PSB, NBLK % PSB))
# gather groups: ramped so the pipeline fills fast, then KGG super-blocks
_ramp = ([1, 1, 2] if int(_osmod.environ.get("KRAMP", "0")) else [])
GGS = []
_i = 0
for _n in _ramp:
    if _i < len(SBS):
        GGS.append(SBS[_i:_i + _n]); _i += _n
while _i < len(SBS):
    GGS.append(SBS[_i:_i + KGG]); _i += KGG
F = 128
FO = 64

# stream offset of cell (block b, chunk k): layout [group][chunk][block]
CELL_OFF = np.zeros((NBLK, CH), np.int64)
GOFF = []   # per group: (stream offset per chunk, first block, nblocks, ntiles)
_base = 0
for _g in GGS:
    _blocks = [b for (b0, nb) in _g for b in range(b0, b0 + nb)]
    _gofs = []
    for _k in range(CH):
        _gofs.append(_base)
        for _b in _blocks:
            CELL_OFF[_b, _k] = _base
            _base += int(CAP[_b])
    GOFF.append((_gofs, _blocks[0], len(_blocks),
                 int(TBLK[_blocks[0]:_blocks[-1] + 1].sum())))
assert _base == NSLOT
MAXNT = max(g[3] for g in GOFF)
NPREB = sum(g[3] for g in GOFF[:KPREB]) * CH

_CACHE = {}


def _pack_core(deg_tot, cnt4, seed=0):
    """Assign the core's NSHARD dsts to NBLK blocks of <=BLK slots so that no
    (block, chunk) cell exceeds CSLOT edges. LPT greedy (largest total degree
    first, block = argmin of projected max cell), then swap-repair."""
    rng = np.random.default_rng(seed)
    order = np.argsort(-deg_tot, kind="stable")
    block_of = np.empty(NSHARD, np.int64)
    loads = np.zeros((NBLK, CH), np.int64)
    counts = np.zeros(NBLK, np.int64)
    for n in order:
        c = cnt4[n]
        key = (loads + c).max(axis=1) * 100000 + loads.sum(axis=1)
        key[counts >= BLK] = 1 << 62
        b = int(np.argmin(key))
        block_of[n] = b
        loads[b] += c
        counts[b] += 1
    cap2 = CAP[:, None]
    for _ in range(8000):
        over = loads - cap2
        mx = over.max()
        if mx <= 0:
            return block_of
        b, j = np.unravel_index(np.argmax(over), loads.shape)
        members = np.where(block_of == b)[0]
        msort = members[np.argsort(-cnt4[members, j])]
        moved = False
        for n in msort[:10]:
            vn = cnt4[n]
            best = None
            for b2 in range(NBLK):
                if b2 == b:
                    continue
                mem2 = np.where(block_of == b2)[0]
                v2 = cnt4[mem2]
                nb = loads[b] - vn[None, :] + v2 - cap2[b]
                nb2 = loads[b2] + vn[None, :] - v2 - cap2[b2]
                s = np.maximum(nb.max(axis=1), nb2.max(axis=1))
                k = int(np.argmin(s))
                if best is None or s[k] < best[0]:
                    best = (s[k], mem2[k], b2)
            if best is not None and best[0] < mx:
                _, n2, b2 = best
                block_of[n], block_of[n2] = b2, b
                loads[b] += cnt4[n2] - vn
                loads[b2] += vn - cnt4[n2]
                moved = True
                break
        if not moved:
            n = rng.choice(members)
            b2 = int(rng.integers(NBLK))
            if b2 == b:
                continue
            mem2 = np.where(block_of == b2)[0]
            n2 = rng.choice(mem2)
            block_of[n], block_of[n2] = b2, b
            loads[b] += cnt4[n2] - cnt4[n]
            loads[b2] += cnt4[n] - cnt4[n2]
    raise RuntimeError("cell packing failed; raise TCELL")


def _host_prep(x, edge_index, batch):
    srcF = edge_index[0].astype(np.int64)
    dstF = edge_index[1].astype(np.int64)
    # degrees include the self-loop (+1); self-loop messages are injected
    # on-device from the local table shard, not via the gather stream
    deg = np.bincount(dstF, minlength=N).astype(np.int64) + 1

    owner_e = dstF // NSHARD
    chunk_e = srcF // NSHARD // 2         # = tablerow(src) // W, packing-free

    # --- pack every core's dsts into blocks ---------------------------------
    block_of_g = np.empty(N, np.int64)
    slot_of_g = np.empty(N, np.int64)
    for c in range(NCORES):
        base = c * NSHARD
        m = owner_e == c
        ed = dstF[m] - base
        cnt4 = np.bincount(
            ed * CH + chunk_e[m], minlength=NSHARD * CH
        ).reshape(NSHARD, CH)
        blk = _pack_core(deg[base : base + NSHARD], cnt4)
        block_of_g[base : base + NSHARD] = blk
        # slot within block: stable order of nodes per block
        o = np.argsort(blk, kind="stable")
        r = np.empty(NSHARD, np.int64)
        r[o] = np.arange(NSHARD) - np.searchsorted(blk[o], blk[o])
        slot_of_g[base : base + NSHARD] = r
        assert r.max() < BLK

    node_owner = np.arange(N) // NSHARD
    tablerow = node_owner * NPAD + block_of_g * BLK + slot_of_g  # per node

    degf = deg.astype(np.float32)
    dstslot = tablerow % BLK              # position of a dst inside its block

    # permuted bf16 x table, shared by all cores
    x_tab = np.zeros((NPAD * NCORES, F), ml_dtypes.bfloat16)
    x_tab[tablerow] = x.astype(ml_dtypes.bfloat16)

    per_core = []
    for c in range(NCORES):
        base = c * NSHARD
        m = owner_e == c
        es, ed = srcF[m], dstF[m]
        eblk = block_of_g[ed]
        idxval = tablerow[es] % W

        cell = eblk * CH + chunk_e[m]
        o = np.argsort(cell, kind="stable")
        cell_s = cell[o]
        cnt = np.bincount(cell_s, minlength=NBLK * CH)
        if (cnt.reshape(NBLK, CH) > CAP[:, None]).any():
            raise RuntimeError("cell overflow; raise profile")
        starts = np.zeros(NBLK * CH, np.int64)
        starts[1:] = np.cumsum(cnt)[:-1]
        rank = np.arange(len(cell_s)) - starts[cell_s]
        pos = CELL_OFF.reshape(-1)[cell_s] + rank

        idxv = np.zeros(NSLOT, np.int16)
        dlv = np.full(NSLOT, -1.0, np.float32)
        dgv = np.ones(NSLOT, np.float32)
        idxv[pos] = idxval[o].astype(np.int16)
        dlv[pos] = dstslot[ed[o]].astype(np.float32)
        dgv[pos] = degf[es[o]]

        core = {}
        wrapped = np.ascontiguousarray(idxv.reshape(-1, 16).T)  # [16, NSLOT/16]
        core["idx"] = np.tile(wrapped, (8, 1))                  # [128, NSLOT/16]
        core["dl"] = np.ascontiguousarray(dlv.reshape(-1, 128).T)  # [128,NTILES]
        core["dg"] = np.ascontiguousarray(dgv.reshape(-1, 128).T)

        # per-slot node metadata in [slot%128, slot//128] layout
        nodes = np.arange(base, base + NSHARD)
        slotidx = block_of_g[nodes] * BLK + slot_of_g[nodes]
        degd = np.ones(NPAD, np.float32)
        degd[slotidx] = degf[nodes]
        blv = np.full(NPAD, -1.0, np.float32)
        blv[slotidx] = batch[nodes].astype(np.float32)
        core["degd"] = np.ascontiguousarray(degd.reshape(NSUB, 128).T)
        core["bl"] = np.ascontiguousarray(blv.reshape(NSUB, 128).T)
        xp_ = x_tab[c * NPAD : (c + 1) * NPAD]          # [NPAD, F]
        core["x_perm"] = np.ascontiguousarray(
            xp_.reshape(NSUB, 128, F).transpose(1, 0, 2).reshape(128, NPAD))
        per_core.append(core)

    return per_core, x_tab


def _build_bass():
    from concourse import bacc, tile, bass
    import concourse.mybir as mybir

    F32 = mybir.dt.float32
    BF16 = mybir.dt.bfloat16
    I16 = mybir.dt.int16
    EQ = mybir.AluOpType.is_equal
    MULT = mybir.AluOpType.mult
    ADD = mybir.AluOpType.add
    MAX = mybir.AluOpType.max
    AF = mybir.ActivationFunctionType

    nc = bacc.Bacc("TRN2", target_bir_lowering=False, debug=False,
                   num_devices=NCORES)

    x_tab = nc.dram_tensor("x_tab", [NPAD * NCORES, F], BF16,
                           kind="ExternalInput")
    x_perm_d = nc.dram_tensor("x_perm", [128, NPAD], BF16, kind="ExternalInput")
    pcol_d = nc.dram_tensor("pcol", [128, 1], F32, kind="ExternalInput")
    idx_d = nc.dram_tensor("idx", [128, NSLOT // 16], I16,
                           kind="ExternalInput")
    dl_d = nc.dram_tensor("dl", [128, NTILES], F32, kind="ExternalInput")
    dg_d = nc.dram_tensor("dg", [128, NTILES], F32, kind="ExternalInput")
    iota_d = nc.dram_tensor("iota", [128, 256], BF16, kind="ExternalInput")
    degd_d = nc.dram_tensor("degd", [128, NSUB], F32, kind="ExternalInput")
    bl_d = nc.dram_tensor("bl", [128, NSUB], F32, kind="ExternalInput")
    w_d = [nc.dram_tensor(f"w{i+1}", [F, F], BF16, kind="ExternalInput")
           for i in range(2)]
    bbc_d = [nc.dram_tensor(f"b{i+1}bc", [128, F], F32, kind="ExternalInput")
             for i in range(2)]
    wmu_d = nc.dram_tensor("wmu", [F, FO], BF16, kind="ExternalInput")
    wlv_d = nc.dram_tensor("wlv", [F, FO], BF16, kind="ExternalInput")
    bmu_d = nc.dram_tensor("bmubc", [128, FO], F32, kind="ExternalInput")
    blv_d = nc.dram_tensor("blvbc", [128, FO], F32, kind="ExternalInput")
    cnt_d = nc.dram_tensor("cnt", [128, 2], F32, kind="ExternalInput")

    mu_o = nc.dram_tensor("mu", [G, FO], F32, kind="ExternalOutput")
    lv_o = nc.dram_tensor("lv", [G, FO], F32, kind="ExternalOutput")

    with tile.TileContext(nc) as tc:
        with (
            tc.tile_pool(name="const", bufs=1) as cp,
            tc.tile_pool(name="stream", bufs=KMSGBUFS) as sp,
            tc.tile_pool(name="work", bufs=KWPBUFS) as wp,
            tc.tile_pool(name="vhp", bufs=KVHBUFS) as vp,
            tc.tile_pool(name="vpre", bufs=max(NPREB, 1)) as vpre,
            tc.tile_pool(name="php", bufs=KPHBUFS) as php,
            tc.tile_pool(name="psum", bufs=KGEMBUFS, space="PSUM") as pp,
            tc.tile_pool(name="psum3", bufs=KAGGBUFS, space="PSUM") as pp3,
            tc.tile_pool(name="psum1", bufs=1, space="PSUM") as pp1,
            tc.tile_pool(name="dram", bufs=1, space="DRAM") as dp,
        ):
            # ---- constants; ordered so the gather/vh path unblocks first ---
            iota = cp.tile([128, 256], BF16, tag="iota")
            nc.sync.dma_start(iota[:], iota_d[:])
            pcol = cp.tile([128, 1], F32, tag="pcol")
            nc.sync.dma_start(pcol[:], pcol_d[:])
            idx_tiles = []

            def load_idx_tiles(sel, eng=None):
                for _gi in sel:
                    _gofs, _b0g, _nbg, _ntg = GOFF[_gi]
                    lo = _gofs[0] // 16
                    hi = (_gofs[CH - 1] + _ntg * 128) // 16
                    it = cp.tile([128, hi - lo], I16, tag=f"idxg{_gi}",
                                 name=f"idxg{_gi}")
                    (eng or nc.sync).dma_start(it[:], idx_d[:, lo:hi])
                    idx_tiles.append((it, lo))

            load_idx_tiles(range(0, 2))
            dl_sb = cp.tile([128, NTILES], F32, tag="dl")
            nc.sync.dma_start(dl_sb[:], dl_d[:])
            # per-edge v = 1/sqrt(max(deg_src,1)), shared by both convs
            dg = cp.tile([128, NTILES], F32, tag="dg")
            nc.sync.dma_start(dg[:], dg_d[:])
            v_sb = cp.tile([128, NTILES], F32, tag="v")
            nc.vector.tensor_scalar(dg[:], dg[:], 1.0, None, MAX)
            nc.scalar.activation(dg[:], dg[:], AF.Sqrt)
            nc.vector.reciprocal(v_sb[:], dg[:])

            zeros = cp.tile([128, 512], BF16, tag="zeros")
            nc.vector.memset(zeros[:], 0.0)
            # bulk uploads are emitted mid-conv1 (after the first gather
            # groups) so they don't hog the DMA engines at startup
            x_sb = cp.tile([128, NPAD], BF16, tag="xsb")

            def emit_late_consts():
                nc.sync.dma_start(x_sb[:], x_perm_d[:])
                load_idx_tiles(range(2, len(GOFF)))
            w_sb = [cp.tile([F, F], BF16, tag=f"w{i}", name=f"w{i}")
                    for i in range(2)]
            bbc_sb = [cp.tile([128, F], F32, tag=f"bbc{i}", name=f"bbc{i}")
                      for i in range(2)]
            for i in range(2):
                nc.sync.dma_start(w_sb[i][:], w_d[i][:])
                nc.sync.dma_start(bbc_sb[i][:], bbc_d[i][:])

            # dinv over the dst shard: 1/sqrt(max(deg,1))
            degd = cp.tile([128, NSUB], F32, tag="degd")
            nc.sync.dma_start(degd[:], degd_d[:])
            dinvd = cp.tile([128, NSUB], F32, tag="dinvd")
            nc.vector.tensor_scalar(degd[:], degd[:], 1.0, None, MAX)
            nc.scalar.activation(degd[:], degd[:], AF.Sqrt)
            nc.vector.reciprocal(dinvd[:], degd[:])

            bl_sb = cp.tile([128, NSUB], F32, tag="bl")
            nc.sync.dma_start(bl_sb[:], bl_d[:])

            wmu = cp.tile([F, FO], BF16, tag="wmu")
            wlv = cp.tile([F, FO], BF16, tag="wlv")
            bmu = cp.tile([128, FO], F32, tag="bmu")
            blv = cp.tile([128, FO], F32, tag="blv")
            for t, d in [(wmu, wmu_d), (wlv, wlv_d), (bmu, bmu_d), (blv, blv_d)]:
                nc.sync.dma_start(t[:], d[:])

            # cnt -> 1/max(cnt,1)
            cnt = cp.tile([128, 2], F32, tag="cnt")
            nc.sync.dma_start(cnt[:], cnt_d[:])
            rcnt = cp.tile([128, 2], F32, tag="rcnt")
            nc.vector.tensor_scalar(cnt[:], cnt[:], 1.0, None, MAX)
            nc.vector.reciprocal(rcnt[:], cnt[:])

            # conv1 output tiles stay resident: conv2 self-loop reads SBUF
            h1_sb = cp.tile([128, NPAD], BF16, tag="h1sb")

            # ---- DRAM intermediates ---------------------------------------
            h1_shard = dp.tile([NPAD, F], BF16)
            h1_full = dp.tile([NPAD * NCORES, F], BF16)
            sums_in = dp.tile([128, 256], BF16)
            sums_out = dp.tile([128, 256], BF16)

            pool_ps = pp1.tile([128, 256], F32, tag="pool", name="pool_ps")
            vh_count = [0]

            def emit_vh(pool, col):
                vh = pool.tile([128, BLK], BF16, tag="vh")
                eng = (nc.gpsimd if KPOOLVH and
                       vh_count[0] % KPOOLVH == KPOOLVH - 1
                       else nc.vector)
                vh_count[0] += 1
                eng.tensor_scalar(
                    vh[:], iota[:, :BLK],
                    dl_sb[:, col : col + 1],
                    v_sb[:, col : col + 1], EQ, MULT,
                )
                return vh

            def issue_gathers(gi, table, gofs, nbg, ntg):
                it, lo = idx_tiles[gi]
                msgs = []
                for k in range(CH):
                    clen = ntg * 128
                    msg = sp.tile([128, MAXNT, F], BF16, tag="msg")
                    nc.gpsimd.dma_gather(
                        msg[:, : ntg, :],
                        table[W * k :, :],
                        it[:, gofs[k] // 16 - lo : (gofs[k] + clen) // 16 - lo],
                        clen, clen, F, elem_step=F,
                        single_packet=False,
                    )
                    msgs.append(msg.rearrange("p t f -> p (t f)"))
                return msgs

            def process_group(conv, msgs, b0g, nbg, ntg, selftab, writer,
                              prebuilt=None):
                first_sb = next(i for i, (b0, nb) in enumerate(SBS)
                                if b0 == b0g)
                n_sbs = (nbg + PSB - 1) // PSB
                for si in range(first_sb, first_sb + n_sbs):
                    b0, nb = SBS[si]
                    agg = pp3.tile([128, 512], F32, tag="agg")
                    # HW: start=True clears has_written for the WHOLE psum
                    # bank — one full-width start matmul per bank.
                    nc.tensor.matmul(agg[:], zeros[:, :128], zeros[:],
                                     start=True, stop=False)
                    for k in range(CH):
                        m2 = msgs[k]
                        for bi in range(nb):
                            b = b0 + bi
                            # tile offset of block b within its group stream
                            tofs = int(TBLK[b0g:b].sum())
                            for t in range(int(TBLK[b])):
                                tl = tofs + t
                                col = CELL_OFF[b, k] // 128 + t
                                if prebuilt is not None and col in prebuilt:
                                    vh = prebuilt[col]
                                else:
                                    vh = emit_vh(vp, col)
                                nc.tensor.matmul(
                                    agg[:, bi * BLK : (bi + 1) * BLK],
                                    m2[:, tl * 128 : (tl + 1) * 128],
                                    vh[:],
                                    start=False, stop=False,
                                )
                    # self-loop term per 128-sub-block (last: selftab for
                    # conv1 is a late upload, for conv2 the conv1 output):
                    # agg[:, sub] += selftab_block^T @ diag(dinv)
                    nsub_sb = nb * BLK // 128
                    for sub in range(nsub_sb):
                        b128 = b0 * (BLK // 128) + sub
                        xl = selftab[:, b128 * 128 : (b128 + 1) * 128]
                        diag = wp.tile([128, 128], BF16, tag="diag")
                        nc.vector.tensor_scalar(
                            diag[:], iota[:, :128], pcol[:],
                            dinvd[:, b128 : b128 + 1], EQ, MULT,
                        )
                        nc.tensor.matmul(
                            agg[:, sub * 128 : (sub + 1) * 128],
                            xl, diag[:], start=False,
                            stop=(sub == nsub_sb - 1),
                        )
                    aggT = wp.tile([128, 512], BF16, tag="aggT")
                    nc.scalar.activation(
                        aggT[:, : nb * BLK], agg[:, : nb * BLK], AF.Copy
                    )
                    for sub in range(nb * BLK // 128):
                        b128 = b0 * (BLK // 128) + sub
                        gm = pp.tile([128, F], F32, tag="gemm")
                        nc.tensor.matmul(
                            gm[:], aggT[:, sub * 128 : (sub + 1) * 128],
                            w_sb[conv][:], start=True, stop=True,
                        )
                        writer(b128, gm)

            def run_conv(conv, table, selftab, writer):
                prebuilt = None
                if conv == 1 and KPREB > 0:
                    # build the first KPREB groups' one-hots BEFORE any
                    # conv2 gather so they fill the AllGather window
                    prebuilt = {}
                    for gofs, b0g, nbg, ntg in GOFF[:KPREB]:
                        for k in range(CH):
                            for b in range(b0g, b0g + nbg):
                                for t in range(int(TBLK[b])):
                                    col = CELL_OFF[b, k] // 128 + t
                                    prebuilt[col] = emit_vh(vpre, col)
                pend = []
                for gi, (gofs, b0g, nbg, ntg) in enumerate(GOFF):
                    msgs = issue_gathers(gi, table, gofs, nbg, ntg)
                    if conv == 0 and gi == 1:
                        emit_late_consts()
                    pend.append((msgs, b0g, nbg, ntg))
                    if len(pend) > KPREFETCH:
                        process_group(conv, *pend.pop(0), selftab, writer,
                                      prebuilt)
                for pg in pend:
                    process_group(conv, *pg, selftab, writer, prebuilt)

            def w_conv1(b, gm):
                h = wp.tile([128, F], F32, tag="h")
                nc.vector.scalar_tensor_tensor(
                    h[:], gm[:], dinvd[:, b : b + 1], bbc_sb[0][:], MULT, ADD,
                )
                hb = h1_sb[:, b * 128 : (b + 1) * 128]
                nc.scalar.activation(hb, h[:], AF.Relu)
                nc.sync.dma_start(h1_shard[b * 128 : (b + 1) * 128, :], hb)

            def w_conv2(b, gm):
                h = wp.tile([128, F], F32, tag="h")
                nc.vector.scalar_tensor_tensor(
                    h[:], gm[:], dinvd[:, b : b + 1], bbc_sb[1][:], MULT, ADD,
                )
                hb = wp.tile([128, F], BF16, tag="hb")
                nc.scalar.activation(hb[:], h[:], AF.Relu)
                ph = php.tile([128, 256], BF16, tag="ph")
                nc.vector.tensor_scalar(
                    ph[:], iota[:], bl_sb[:, b : b + 1], None, EQ,
                )
                nc.tensor.matmul(
                    pool_ps[:], hb[:], ph[:],
                    start=(b == 0), stop=(b == NSUB - 1),
                )

            run_conv(0, x_tab, x_sb, w_conv1)

            # conv1 writes only a per-core shard; gather it for conv2's table
            nc.gpsimd.collective_compute(
                "AllGather", mybir.AluOpType.bypass,
                replica_groups=[list(range(NCORES))],
                ins=[h1_shard.opt()], outs=[h1_full.opt()],
            )
            run_conv(1, h1_full, h1_sb, w_conv2)

            # ---- pooling sums AllReduce + heads ---------------------------
            pool_sb = wp.tile([128, 256], BF16, tag="poolsb")
            nc.vector.tensor_copy(pool_sb[:], pool_ps[:])
            nc.sync.dma_start(sums_in[:], pool_sb[:])
            nc.gpsimd.collective_compute(
                "AllReduce", mybir.AluOpType.add,
                replica_groups=[list(range(NCORES))],
                ins=[sums_in.opt()], outs=[sums_out.opt()],
            )
            sums_sb = wp.tile([128, 256], BF16, tag="sums")
            nc.sync.dma_start(sums_sb[:], sums_out[:])
            outq = [nc.sync, nc.scalar, nc.gpsimd, nc.scalar]
            qi = 0
            for j in range(2):
                for wt, bt, out_d in [(wmu, bmu, mu_o), (wlv, blv, lv_o)]:
                    hp = pp.tile([128, FO], F32, tag="head")
                    nc.tensor.matmul(
                        hp[:], sums_sb[:, j * 128 : (j + 1) * 128], wt[:],
                        start=True, stop=True,
                    )
                    hs = wp.tile([128, FO], F32, tag="headsb")
                    nc.vector.scalar_tensor_tensor(
                        hs[:], hp[:], rcnt[:, j : j + 1], bt[:], MULT, ADD,
                    )
                    outq[qi % 4].dma_start(
                        out_d[j * 128 : (j + 1) * 128, :], hs[:])
                    qi += 1

    nc.compile()
    return nc


def kernel(x, edge_index, batch, W1, b1, W2, b2, W_mu, b_mu, W_lv, b_lv):
    from concourse import bass_utils

    x = np.asarray(x, dtype=np.float32)
    edge_index = np.asarray(edge_index)
    batch = np.asarray(batch)

    per_core, x_tab = _host_prep(x, edge_index, batch)

    iota = np.broadcast_to(
        np.arange(256, dtype=np.float32), (128, 256)
    ).astype(ml_dtypes.bfloat16).copy()
    cnts = np.bincount(np.asarray(batch, np.int64), minlength=G).astype(np.float32)
    cnt_arr = np.ascontiguousarray(cnts.reshape(2, 128).T)
    shared = dict(
        x_tab=x_tab,
        iota=iota,
        pcol=np.arange(128, dtype=np.float32).reshape(128, 1),
        w1=np.asarray(W1, np.float32).astype(ml_dtypes.bfloat16),
        w2=np.asarray(W2, np.float32).astype(ml_dtypes.bfloat16),
        b1bc=np.broadcast_to(np.asarray(b1, np.float32), (128, F)).copy(),
        b2bc=np.broadcast_to(np.asarray(b2, np.float32), (128, F)).copy(),
        wmu=np.asarray(W_mu, np.float32).astype(ml_dtypes.bfloat16),
        wlv=np.asarray(W_lv, np.float32).astype(ml_dtypes.bfloat16),
        bmubc=np.broadcast_to(np.asarray(b_mu, np.float32), (128, FO)).copy(),
        blvbc=np.broadcast_to(np.asarray(b_lv, np.float32), (128, FO)).copy(),
        cnt=cnt_arr,
    )
    in_maps = [dict(shared, **pc) for pc in per_core]

    if "nc" not in _CACHE:
        _CACHE["nc"] = _build_bass()
    nc = _CACHE["nc"]

    import os as _os
    res = bass_utils.run_bass_kernel_spmd(
        nc, in_maps, core_ids=list(range(NCORES)),
        trace=_os.environ.get("KTRACE") == "1",
    )
    _CACHE["last_res"] = res
    r0 = res.results[0]
    return (r0["mu"].copy(), r0["lv"].copy())

